# revision 30
# baseline (speedup 1.0000x reference)
"""LoRA-MoE layer (base dense + top-2 routed rank-16 LoRA experts) on 8 TRN2 cores.

Strategy: data-parallel over tokens (8192 tokens -> 1024/core), all weights
replicated, zero collectives. Per-core fused Bass/Tile kernel.

Key idea vs the v0 kernel: the router logits and the LoRA-A projection ride
the SAME stationary x-tile as the base matmul. For each (token-tile ti, k)
the PE loads xh[ti,k] [128x128] once and streams [W-half (1024 cols) |
Rh|A (136 cols)], so routing+u cost ~7% extra streaming instead of
separate full passes over x. Logits and u come out in [token, *] layout,
which kills all the logit transposes; the top-2 softmax chain runs directly
on [t,8] tiles while the next token-tile's matmuls keep the PE busy.

The output O-dim is split in two sweeps (o<1024 with the aux stream, then
o>=1024) so PSUM fits: base pool 6 banks, aux accumulators (which also hold the
transpose scratch in their upper columns) 2 banks = 8.

Precision: x and all weights in bf16 (f32 PSUM accumulation), including
the router (measured end-to-end rel err 4.4e-3 vs 2e-2 budget; 16/8192
tokens flip top-2 vs f32 routing).
"""

import os
import sys

import numpy as np


def _ensure_concourse():
    try:
        import concourse  # noqa: F401
    except ImportError:
        for p in ("/opt/trn_rl_repo", os.path.expanduser("~/.axon_site/_ro/trn_rl_repo")):
            if os.path.isdir(p):
                sys.path.insert(0, p)
                break


_ensure_concourse()

import ml_dtypes  # noqa: E402
import concourse.bass as bass  # noqa: E402,F401
import concourse.tile as tile  # noqa: E402
from concourse import bacc, mybir  # noqa: E402

F32 = mybir.dt.float32
BF16 = mybir.dt.bfloat16
X_AX = mybir.AxisListType.X
ALU = mybir.AluOpType
ACT = mybir.ActivationFunctionType

N_CORES = 8
N_TOK = 8192          # total tokens (4 x 2048)
NT = N_TOK // N_CORES  # tokens per core = 1024
D = 2048
O = 2048
E = 8
R = 16
ER = E * R            # 128
KT = D // 128         # 16 contraction chunks
TI = NT // 128        # 8 token tiles
RA_W = E + ER         # 136 aux stream columns: Rh(8) | A(128)

_NC_CACHE = {}
LAST_RESULTS = None


def _body(tc, nc, xh, W0, W1, RA, Bc, Idb, out):
    with (
        tc.tile_pool(name="const", bufs=1) as constp,
        tc.tile_pool(name="small", bufs=2) as smallp,
        tc.tile_pool(name="stage", bufs=4) as stagep,
        tc.tile_pool(name="ps_base", bufs=6, space="PSUM") as psbasep,
        tc.tile_pool(name="ps_aux", bufs=2, space="PSUM") as psauxp,
    ):
        # ---- resident SBUF tensors ----
        # xh is 3D so the per-ti DMA lowers to one contiguous 4KB line per
        # partition (a 4D [:, ti, :, :] slice emits 16x256B packets instead).
        xh_sb = constp.tile([128, TI, KT * 128], BF16, name="xh_sb")
        W0_sb = constp.tile([128, KT, 1024], BF16, name="W0_sb")
        W1_sb = constp.tile([128, KT, 1024], BF16, name="W1_sb")
        RA_sb = constp.tile([128, KT, RA_W], BF16, name="RA_sb")
        Bc_sb = constp.tile([ER, O], BF16, name="Bc_sb")
        Idb_sb = constp.tile([128, 128], F32, name="Idb_sb")
        usT_sb = constp.tile([ER, NT], BF16, name="usT_sb")

        # DMA priority: aux weights + first x tile pace the start; all of
        # W-half0 next (it paces ti0/ti1's k-loops); remaining x tiles and
        # Bc after; W-half1 last (needed only from sweep1, ~70us in).
        # Transfers are runtime-striped across all 16 queues, so fewer,
        # larger dma_starts cost nothing in pacing but shrink the per-DMA
        # semaphore set (teardown resets every semaphore serially).
        HX = KT * 128 // 2
        nc.sync.dma_start(RA_sb[:], RA[:])
        nc.sync.dma_start(xh_sb[:, 0, 0:256], xh[:, 0, 0:256])
        nc.sync.dma_start(xh_sb[:, 1, 0:256], xh[:, 1, 0:256])
        nc.sync.dma_start(W0_sb[:, 0, 0:512], W0[:, 0, 0:512])
        nc.sync.dma_start(W0_sb[:, 0, 512:1024], W0[:, 0, 512:1024])
        nc.sync.dma_start(W0_sb[:, 1, :], W0[:, 1, :])
        nc.sync.dma_start(xh_sb[:, 0, 256:HX], xh[:, 0, 256:HX])
        nc.sync.dma_start(xh_sb[:, 1, 256:HX], xh[:, 1, 256:HX])
        nc.sync.dma_start(W0_sb[:, 2, :], W0[:, 2, :])
        nc.sync.dma_start(W0_sb[:, 3, :], W0[:, 3, :])
        nc.sync.dma_start(xh_sb[:, 0, HX:], xh[:, 0, HX:])
        nc.sync.dma_start(xh_sb[:, 1, HX:], xh[:, 1, HX:])
        nc.sync.dma_start(Idb_sb[:], Idb[:])
        for b in range(2, KT // 2):
            nc.sync.dma_start(W0_sb[:, 2 * b:2 * b + 2, :], W0[:, 2 * b:2 * b + 2, :])
        for ti in range(2, TI):
            nc.sync.dma_start(xh_sb[:, ti, :], xh[:, ti, :])
            if ti == 2:
                nc.sync.dma_start(Bc_sb[:], Bc[:])
        nc.sync.dma_start(W1_sb[:, 0:8, :], W1[:, 0:8, :])
        nc.sync.dma_start(W1_sb[:, 8:16, :], W1[:, 8:16, :])

        aux_tiles = [None] * TI
        base_tiles = [None] * TI

        def alloc0(ti):
            aux_tiles[ti] = psauxp.tile([128, 512], F32, name=f"aux{ti}", tag="aux")
            base_tiles[ti] = [
                psbasep.tile([128, 512], F32, name=f"b{ti}_{j}", tag="mm")
                for j in range(2)
            ]

        def kgroup0(ti, k):
            st = xh_sb[:, ti, k * 128:(k + 1) * 128]
            nc.tensor.matmul(aux_tiles[ti][:, 0:RA_W], st, RA_sb[:, k, :],
                             start=(k == 0), stop=(k == KT - 1))
            nc.tensor.matmul(base_tiles[ti][0][:], st, W0_sb[:, k, 0:512],
                             start=(k == 0), stop=False)
            nc.tensor.matmul(base_tiles[ti][1][:], st, W0_sb[:, k, 512:1024],
                             start=(k == 0), stop=False)

        def kloop0(ti):
            alloc0(ti)
            for k in range(KT):
                kgroup0(ti, k)

        def evict_pair(bpair, ti, obase, split=False):
            # one staging tile + one striped DMA per (ti, sweep): 4KB HBM
            # lines and half the dma_start/semaphore count of per-bank evicts.
            # split=True (used for the final tile, where the DMA is the tail)
            # issues per-half DMAs so transfer overlaps the second copy.
            st = stagep.tile([128, 1024], F32, name="st", tag="st")
            tsl = slice(ti * 128, (ti + 1) * 128)
            nc.scalar.copy(st[:, 0:512], bpair[0][:])
            if split:
                nc.sync.dma_start(out[tsl, obase:obase + 512], st[:, 0:512])
            nc.vector.tensor_copy(st[:, 512:1024], bpair[1][:])
            if split:
                nc.sync.dma_start(out[tsl, obase + 512:obase + 1024], st[:, 512:1024])
            else:
                nc.sync.dma_start(out[tsl, obase:obase + 1024], st[:])

        def post0_a(ti):
            aux_ps = aux_tiles[ti]
            tsl = slice(ti * 128, (ti + 1) * 128)
            # top-2 softmax weights from the logits in aux cols 0:E
            L = smallp.tile([128, E], F32, name="L", tag="L")
            nc.scalar.copy(L[:], aux_ps[:, 0:E])
            m1 = smallp.tile([128, 1], F32, name="m1", tag="m1")
            nc.vector.reduce_max(m1[:], L[:], axis=X_AX)
            nm1 = smallp.tile([128, 1], F32, name="nm1", tag="nm1")
            nc.scalar.mul(nm1[:], m1[:], -1.0)
            msk = smallp.tile([128, E], F32, name="msk", tag="msk")
            nc.vector.tensor_scalar(msk[:], L[:], m1[:], -1e30, ALU.is_equal, ALU.mult)
            L2 = smallp.tile([128, E], F32, name="L2", tag="L2")
            nc.vector.tensor_tensor(L2[:], L[:], msk[:], ALU.add)
            m2 = smallp.tile([128, 1], F32, name="m2", tag="m2")
            nc.vector.reduce_max(m2[:], L2[:], axis=X_AX)
            eL = smallp.tile([128, E], F32, name="eL", tag="eL")
            nc.scalar.activation(eL[:], L[:], ACT.Exp, bias=nm1[:])
            ge = smallp.tile([128, E], F32, name="ge", tag="ge")
            nc.vector.tensor_scalar(ge[:], L[:], m2[:], None, ALU.is_ge)
            un = smallp.tile([128, E], F32, name="un", tag="un")
            nc.vector.tensor_tensor(un[:], eL[:], ge[:], ALU.mult)
            s = smallp.tile([128, 1], F32, name="s", tag="s")
            nc.vector.reduce_sum(s[:], un[:], axis=X_AX)
            r = smallp.tile([128, 1], F32, name="r", tag="r")
            nc.vector.reciprocal(r[:], s[:])
            r2 = smallp.tile([128, 1], F32, name="r2", tag="r2")
            nc.scalar.mul(r2[:], r[:], 2.0)  # fold SCALING = 2.0
            w = smallp.tile([128, E], F32, name="w", tag="w")
            nc.vector.tensor_scalar(w[:], un[:], r2[:], None, ALU.mult)
            # us[t, er] = u[t, er] * w[t, e(er)]  (u lives in aux psum cols 8:136)
            us_ter = smallp.tile([128, ER], F32, name="us_ter", tag="us_ter")
            for e in range(E):
                esl = slice(e * R, (e + 1) * R)
                nc.vector.tensor_scalar(
                    us_ter[:, esl], aux_ps[:, E + e * R:E + (e + 1) * R],
                    w[:, e:e + 1], None, ALU.mult,
                )
            # transpose to [er, t] for the expert matmul's stationary operand;
            # scratch packs into the aux tile's free upper columns (same bank)
            nc.tensor.transpose(aux_ps[:, 256:384], us_ter[:], Idb_sb[:])
            nc.scalar.copy(usT_sb[:, tsl], aux_ps[:, 256:384])

        def post0_b(ti):
            # expert contribution for o-half0 + eviction
            tsl = slice(ti * 128, (ti + 1) * 128)
            b = base_tiles[ti]
            for j in range(2):
                osl = slice(j * 512, (j + 1) * 512)
                nc.tensor.matmul(b[j][:], usT_sb[:, tsl], Bc_sb[:, osl],
                                 start=False, stop=True)
            evict_pair(b, ti, 0)

        def post0(ti):
            post0_a(ti)
            post0_b(ti)

        # ---- sweep 0: o in [0, 1024) + routing/aux ----
        # ti0+ti1 run as one fused k-major loop: the PE consumes fresh W0
        # faster than HBM delivers it (525 vs ~390 GB/s), so each W0 chunk
        # is streamed for two token tiles while it is hot; later tiles hit
        # resident W0 and run tile-major.
        alloc0(0)
        alloc0(1)
        for k in range(KT):
            kgroup0(0, k)
            kgroup0(1, k)
        post0(0)
        for ti in range(2, TI):
            alloc0(ti)
            for k in range(KT - 1):
                kgroup0(ti, k)
            post0_a(ti - 1)
            kgroup0(ti, KT - 1)
            post0_b(ti - 1)

        # ---- sweep 1: o in [1024, 2048), routing known ----
        # post0(7) is delayed into sweep1 so its softmax chain overlaps
        # kloop1(0) instead of stalling the PE at the sweep boundary.
        for ti in range(TI - 1):
            if ti == 1:
                post0(TI - 1)
            tsl = slice(ti * 128, (ti + 1) * 128)
            c = [
                psbasep.tile([128, 512], F32, name=f"c{ti}_{j}", tag="mm")
                for j in range(2)
            ]
            for k in range(KT):
                st = xh_sb[:, ti, k * 128:(k + 1) * 128]
                nc.tensor.matmul(c[0][:], st, W1_sb[:, k, 0:512],
                                 start=(k == 0), stop=False)
                nc.tensor.matmul(c[1][:], st, W1_sb[:, k, 512:1024],
                                 start=(k == 0), stop=False)
            for j in range(2):
                osl = slice(1024 + j * 512, 1024 + (j + 1) * 512)
                nc.tensor.matmul(c[j][:], usT_sb[:, tsl], Bc_sb[:, osl],
                                 start=False, stop=True)
            evict_pair(c, ti, 1024)

        # Final tile runs as two sequential half-o k-loops so the first
        # half's eviction DMA overlaps the second half's matmuls; only
        # 256KB of output remains after the very last matmul.
        ti = TI - 1
        tsl = slice(ti * 128, (ti + 1) * 128)
        for j in range(2):
            osl = slice(1024 + j * 512, 1024 + (j + 1) * 512)
            cj = psbasep.tile([128, 512], F32, name=f"cl{j}", tag="mm")
            for k in range(KT):
                st = xh_sb[:, ti, k * 128:(k + 1) * 128]
                nc.tensor.matmul(cj[:], st, W1_sb[:, k, j * 512:(j + 1) * 512],
                                 start=(k == 0), stop=False)
            nc.tensor.matmul(cj[:], usT_sb[:, tsl], Bc_sb[:, osl],
                             start=False, stop=True)
            stl = stagep.tile([128, 1024], F32, name="st", tag="st")
            if j == 0:
                nc.scalar.copy(stl[:, 0:512], cj[:])
            else:
                nc.vector.tensor_copy(stl[:, 0:512], cj[:])
            nc.sync.dma_start(out[tsl, osl], stl[:, 0:512])


def build_nc():
    nc = bacc.Bacc("TRN2", target_bir_lowering=False, debug=False, num_devices=N_CORES)
    xh = nc.dram_tensor("xh", [128, TI, KT * 128], BF16, kind="ExternalInput").ap()
    W0 = nc.dram_tensor("W0", [128, KT, 1024], BF16, kind="ExternalInput").ap()
    W1 = nc.dram_tensor("W1", [128, KT, 1024], BF16, kind="ExternalInput").ap()
    RA = nc.dram_tensor("RA", [128, KT, RA_W], BF16, kind="ExternalInput").ap()
    Bc = nc.dram_tensor("Bc", [ER, O], BF16, kind="ExternalInput").ap()
    Idb = nc.dram_tensor("Idb", [128, 128], F32, kind="ExternalInput").ap()
    out = nc.dram_tensor("out", [NT, O], F32, kind="ExternalOutput").ap()
    with tile.TileContext(nc) as tc:
        _body(tc, nc, xh, W0, W1, RA, Bc, Idb, out)
    nc.compile()
    return nc


def get_nc():
    if "nc" not in _NC_CACHE:
        _NC_CACHE["nc"] = build_nc()
    return _NC_CACHE["nc"]


def _bf16(a):
    return a.astype(ml_dtypes.bfloat16)


def make_in_maps(x, weight, lora_A, lora_B, router_w):
    x = np.ascontiguousarray(np.asarray(x, dtype=np.float32)).reshape(N_TOK, D)
    weight = np.asarray(weight, dtype=np.float32)
    lora_A = np.asarray(lora_A, dtype=np.float32)
    lora_B = np.asarray(lora_B, dtype=np.float32)
    router_w = np.asarray(router_w, dtype=np.float32)

    def to_pk(a):
        # [D, C] -> [128, KT, C]: partition p holds row k*128+p for each k chunk
        return np.ascontiguousarray(a.reshape(KT, 128, a.shape[1]).transpose(1, 0, 2))

    WT = np.ascontiguousarray(weight.T).astype(ml_dtypes.bfloat16)
    Wpk = to_pk(WT)                                   # [128, KT, O]
    W0m = np.ascontiguousarray(Wpk[:, :, 0:1024])
    W1m = np.ascontiguousarray(Wpk[:, :, 1024:2048])
    rh = _bf16(np.ascontiguousarray(router_w.T))      # [D, E] bf16
    ATm = _bf16(lora_A.reshape(ER, D).T)              # [D, ER]
    RAm = np.concatenate(
        [to_pk(rh), to_pk(ATm)], axis=2)              # [128, KT, 136]
    Bcm = _bf16(lora_B.transpose(0, 2, 1).reshape(ER, O))
    Idb = np.eye(128, dtype=np.float32)

    in_maps = []
    for c in range(N_CORES):
        xT = _bf16(np.ascontiguousarray(x[c * NT:(c + 1) * NT].T))  # [D, NT] bf16
        # [D, NT] -> [128, TI, KT, 128]: partition p, token-tile ti, chunk k
        xp = np.ascontiguousarray(
            xT.reshape(KT, 128, TI, 128).transpose(1, 2, 0, 3)).reshape(128, TI, KT * 128)
        in_maps.append({
            "xh": xp,
            "W0": W0m,
            "W1": W1m,
            "RA": RAm,
            "Bc": Bcm,
            "Idb": Idb,
        })
    return in_maps


def kernel(x, weight, lora_A, lora_B, router_w):
    global LAST_RESULTS
    from concourse.bass_utils import run_bass_kernel_spmd

    in_maps = make_in_maps(x, weight, lora_A, lora_B, router_w)
    nc = get_nc()
    trace = bool(os.environ.get("KBENCH_TRACE"))
    res = run_bass_kernel_spmd(nc, in_maps, core_ids=list(range(N_CORES)), trace=trace)
    LAST_RESULTS = res
    outs = [np.asarray(res.results[c]["out"], dtype=np.float32) for c in range(N_CORES)]
    return np.concatenate(outs, axis=0).reshape(4, 2048, 2048)
